# revision 6
# baseline (speedup 1.0000x reference)
"""Savitzky-Golay noise-reduction filter (window=11, poly=3) on Trainium2.

Input x: (64, 16, 65536) fp32. Output: same shape, savgol filtered along the
last axis with scipy mode='interp' edge handling (polynomial fit on the first/
last WINDOW samples).

Strategy (pure data parallel, 8 cores, 128 rows of length 65536 per core):
  - Overlapping 128-sample input windows (stride 118) are PE-transposed so
    time lies on partitions, then one fp32 matmul with a banded weight matrix
    computes 118 (123 for edge windows) outputs per window; edge-polynomial
    rows are folded into the first/last windows' weight matrices.
  - PE transpose-back returns natural layout; results are staged in SBUF and
    stored with large contiguous DMAs.
"""

from contextlib import ExitStack

import numpy as np

WINDOW = 11
POLY = 3
HALF = WINDOW // 2  # 5
P = 128
L = 65536
N_CORES = 8
ROWS_TOTAL = 1024  # 64*16
ROWS_PER_CORE = ROWS_TOTAL // N_CORES  # 128
STRIDE = P - (WINDOW - 1)  # 118
EDGE_W = P - HALF  # 123: outputs of first/last window


def _savgol_matrices():
    pos = np.arange(-HALF, HALF + 1, dtype=np.float64)
    A = pos[:, None] ** np.arange(POLY + 1)[None, :]
    c = np.linalg.pinv(A)[0]  # [W] central taps
    V = np.arange(WINDOW, dtype=np.float64)[:, None] ** np.arange(POLY + 1)[None, :]
    T = np.arange(HALF, dtype=np.float64)[:, None] ** np.arange(POLY + 1)[None, :]
    E = T @ np.linalg.pinv(V)  # [HALF, W]
    return c, E


def _build_weights():
    """W matrices [128, 128] (stationary lhsT: W[q, p] multiplies input q for
    output p).  Returns (W_first, W_mid, W_last) float32."""
    c64, E64 = _savgol_matrices()
    c = c64.astype(np.float32)
    E = E64.astype(np.float32)

    w_mid = np.zeros((P, P), np.float32)
    for p in range(STRIDE):  # output p -> y[base + 5 + p] = sum_k c_k x[base+p+k]
        for k in range(WINDOW):
            w_mid[p + k, p] = c[k]

    w_first = np.zeros((P, P), np.float32)
    for p in range(HALF):  # y[p] = sum_w E[p, w] x[w]
        for w in range(WINDOW):
            w_first[w, p] = E[p, w]
    for p in range(HALF, EDGE_W):  # y[p] = sum_k c_k x[p - 5 + k]
        for k in range(WINDOW):
            w_first[p - HALF + k, p] = c[k]

    w_last = np.zeros((P, P), np.float32)
    for p in range(STRIDE):  # y[L-123+p] = sum_k c_k x[base + p + k]
        for k in range(WINDOW):
            w_last[p + k, p] = c[k]
    for h in range(HALF):  # y[L-5+h] = sum_w E[4-h, w] x[L-1-w] = x[base+127-w]
        for w in range(WINDOW):
            w_last[127 - w, STRIDE + h] = E[HALF - 1 - h, w]

    return w_first, w_mid, w_last


def _windows():
    """List of (in_off, out_off, out_width, wtype) per window; wtype in
    {'first','mid','last'}."""
    wins = [(0, 0, EDGE_W, "first")]
    j = 1
    while True:
        in_off = STRIDE * j
        out_off = in_off + HALF
        if out_off + STRIDE >= L - HALF:
            break
        wins.append((in_off, out_off, STRIDE, "mid"))
        j += 1
    wins.append((L - P, L - EDGE_W, EDGE_W, "last"))
    return wins


def reference_rows(x):
    """Numpy reference for [rows, L] (mirrors the jax reference)."""
    c64, E64 = _savgol_matrices()
    c = c64.astype(np.float32)
    E = E64.astype(np.float32)
    R, Lx = x.shape
    out = np.empty_like(x)
    # interior via correlation
    from numpy.lib.stride_tricks import sliding_window_view

    sw = sliding_window_view(x, WINDOW, axis=1)  # [R, L-10, 11]
    out[:, HALF : Lx - HALF] = np.einsum("rlk,k->rl", sw, c, optimize=True).astype(
        np.float32
    )
    out[:, :HALF] = x[:, :WINDOW] @ E.T
    out[:, Lx - HALF :] = (x[:, ::-1][:, :WINDOW] @ E.T)[:, ::-1]
    return out


def simulate_host(x):
    """Pure-numpy simulation of the windowed scheme, to validate W matrices."""
    w_first, w_mid, w_last = _build_weights()
    wmap = {"first": w_first, "mid": w_mid, "last": w_last}
    R, Lx = x.shape
    out = np.zeros_like(x)
    for in_off, out_off, width, t in _windows():
        xw = x[:, in_off : in_off + P]  # [R, 128]
        yw = xw @ wmap[t]  # [R, 128]
        out[:, out_off : out_off + width] = yw[:, :width]
    return out


# ---------------------------------------------------------------------------
# Bass kernel
# ---------------------------------------------------------------------------

IO_BATCH = 32  # windows per DMA in/out batch
CONV_GROUP = 4  # windows per conv matmul (moving dim 4*128=512)

_NC_CACHE = None
TRACE = False  # set by test harness to capture an NTFF profile
LAST_RESULTS = None  # BassKernelResults of the most recent kernel() call


XT_COPY_ENGINE = "vector"  # engine for PSUM->SBUF copy of transposed windows
Y_COPY_ENGINE = "scalar"  # engine for PSUM->SBUF copy of conv outputs


def _build_nc(reps: int = 1, n_batches: int | None = None, ablate: frozenset = frozenset()):
    import concourse.tile as tile
    from concourse import bacc, mybir
    from concourse.masks import make_identity

    nc = bacc.Bacc(
        "TRN2",
        target_bir_lowering=False,
        debug=False,
        enable_asserts=False,
        num_devices=N_CORES,
    )
    bf16 = mybir.dt.bfloat16
    f32 = mybir.dt.float32
    x = nc.dram_tensor("x", [P, L], f32, kind="ExternalInput").ap()
    wf = nc.dram_tensor("w_first", [P, P], bf16, kind="ExternalInput").ap()
    wm = nc.dram_tensor("w_mid", [P, P], bf16, kind="ExternalInput").ap()
    wl = nc.dram_tensor("w_last", [P, P], bf16, kind="ExternalInput").ap()
    y = nc.dram_tensor("y", [P, L], f32, kind="ExternalOutput").ap()

    wins = _windows()
    batches = [wins[i : i + IO_BATCH] for i in range(0, len(wins), IO_BATCH)]
    if n_batches is not None:
        batches = batches[:n_batches]

    with tile.TileContext(nc) as tc:
        with ExitStack() as ctx:
            consts = ctx.enter_context(tc.tile_pool(name="consts", bufs=1))
            in_pool = ctx.enter_context(tc.tile_pool(name="inp", bufs=3))
            out_pool = ctx.enter_context(tc.tile_pool(name="outp", bufs=3))
            xt_pool = ctx.enter_context(tc.tile_pool(name="xt", bufs=4))
            ps_t = ctx.enter_context(tc.tile_pool(name="ps_t", bufs=4, space="PSUM"))
            ps_c = ctx.enter_context(tc.tile_pool(name="ps_c", bufs=4, space="PSUM"))

            ident = consts.tile([P, P], f32, tag="ident")
            make_identity(nc, ident[:])
            wt = {}
            for name, ap in (("first", wf), ("mid", wm), ("last", wl)):
                t = consts.tile([P, P], bf16, tag=f"w_{name}")
                nc.sync.dma_start(t[:], ap)
                wt[name] = t

            def copy(engine, dst, src):
                if engine == "scalar":
                    nc.scalar.copy(dst, src)
                else:
                    nc.vector.tensor_copy(dst, src)

            for _rep in range(reps):
              for batch in batches:
                in_base = batch[0][0]
                in_span = batch[-1][0] + P - in_base
                out_base = batch[0][1]
                out_span = batch[-1][1] + batch[-1][2] - out_base

                xin = in_pool.tile([P, IO_BATCH * STRIDE + 256], f32, tag="xin")
                nc.sync.dma_start(
                    xin[:, :in_span], x[:, in_base : in_base + in_span]
                )
                stag = out_pool.tile([P, IO_BATCH * STRIDE + 256], f32, tag="stag")

                # group windows by consecutive same-type runs of <= CONV_GROUP
                groups = []
                cur = []
                for w in batch:
                    if cur and (w[3] != cur[0][3] or len(cur) == CONV_GROUP):
                        groups.append(cur)
                        cur = []
                    cur.append(w)
                groups.append(cur)

                for grp in groups:
                    g = len(grp)
                    wtype = grp[0][3]
                    # 1) PE transpose each window into one PSUM bank (fp32,
                    #    exact)
                    pt = ps_t.tile([P, 512], f32, tag="pt")
                    for s, (in_off, _, _, _) in enumerate(grp):
                        o = in_off - in_base
                        nc.tensor.transpose(
                            pt[:, s * P : (s + 1) * P],
                            xin[:, o : o + P],
                            ident[:],
                        )
                    # 2) copy PSUM -> SBUF, converting to bf16 (rel tol is
                    #    2e-2; bf16 conv gives ~4e-3)
                    xt = xt_pool.tile([P, 512], bf16, tag="xt")
                    copy(XT_COPY_ENGINE, xt[:, : g * P], pt[:, : g * P])
                    # 3) conv matmul per window in bf16 (1 cycle/row on PE vs
                    #    4 for fp32), stationary = transposed window, moving =
                    #    W -> output lands in NATURAL layout, packed
                    #    contiguously in the PSUM bank
                    width = grp[0][2]
                    pc = ps_c.tile([P, 512], f32, tag="pc")
                    for s in range(g):
                        nc.tensor.matmul(
                            pc[:, s * width : (s + 1) * width],
                            xt[:, s * P : (s + 1) * P],
                            wt[wtype][:, :width],
                            start=True,
                            stop=True,
                        )
                    # 4) single contiguous copy into the output staging buffer
                    soff = grp[0][1] - out_base
                    copy(Y_COPY_ENGINE, stag[:, soff : soff + g * width], pc[:, : g * width])

                nc.sync.dma_start(
                    y[:, out_base : out_base + out_span], stag[:, :out_span]
                )

    nc.compile()
    return nc


def _get_nc():
    global _NC_CACHE
    if _NC_CACHE is None:
        _NC_CACHE = _build_nc()
    return _NC_CACHE


def _make_in_maps(x: np.ndarray) -> list[dict]:
    import ml_dtypes

    w_first, w_mid, w_last = _build_weights()
    w_first = w_first.astype(ml_dtypes.bfloat16)
    w_mid = w_mid.astype(ml_dtypes.bfloat16)
    w_last = w_last.astype(ml_dtypes.bfloat16)
    xr = np.ascontiguousarray(x.reshape(ROWS_TOTAL, L))
    return [
        {
            "x": xr[i * ROWS_PER_CORE : (i + 1) * ROWS_PER_CORE],
            "w_first": w_first,
            "w_mid": w_mid,
            "w_last": w_last,
        }
        for i in range(N_CORES)
    ]


def kernel(x: np.ndarray) -> np.ndarray:
    from concourse.bass_utils import run_bass_kernel_spmd

    assert x.shape == (64, 16, L) and x.dtype == np.float32
    nc = _get_nc()
    in_maps = _make_in_maps(x)
    res = run_bass_kernel_spmd(
        nc, in_maps, core_ids=list(range(N_CORES)), trace=TRACE
    )
    globals()["LAST_RESULTS"] = res
    out = np.concatenate([r["y"] for r in res.results], axis=0)
    return out.reshape(64, 16, L)


if __name__ == "__main__":
    # host-side validation of the window scheme
    rng = np.random.default_rng(0)
    xt = rng.standard_normal((4, L)).astype(np.float32)
    ref = reference_rows(xt)
    sim = simulate_host(xt)
    err = np.abs(sim - ref).max()
    rel = err / np.abs(ref).max()
    print(f"host sim vs ref: max abs {err:.3e}  rel {rel:.3e}")
    print("n windows:", len(_windows()))



# revision 9
# speedup vs baseline: 821.9572x; 821.9572x over previous
"""Savitzky-Golay noise-reduction filter (window=11, poly=3) on Trainium2.

Input x: (64, 16, 65536) fp32. Output: same shape, savgol filtered along the
last axis with scipy mode='interp' edge handling (polynomial fit on the first/
last WINDOW samples).

Strategy (pure data parallel, 8 cores, 128 rows of length 65536 per core):
  - Overlapping 128-sample input windows (stride 118) are PE-transposed so
    time lies on partitions, then one fp32 matmul with a banded weight matrix
    computes 118 (123 for edge windows) outputs per window; edge-polynomial
    rows are folded into the first/last windows' weight matrices.
  - PE transpose-back returns natural layout; results are staged in SBUF and
    stored with large contiguous DMAs.
"""

from contextlib import ExitStack

import numpy as np

WINDOW = 11
POLY = 3
HALF = WINDOW // 2  # 5
P = 128
L = 65536
N_CORES = 8
ROWS_TOTAL = 1024  # 64*16
ROWS_PER_CORE = ROWS_TOTAL // N_CORES  # 128
STRIDE = P - (WINDOW - 1)  # 118
EDGE_W = P - HALF  # 123: outputs of first/last window


def _savgol_matrices():
    pos = np.arange(-HALF, HALF + 1, dtype=np.float64)
    A = pos[:, None] ** np.arange(POLY + 1)[None, :]
    c = np.linalg.pinv(A)[0]  # [W] central taps
    V = np.arange(WINDOW, dtype=np.float64)[:, None] ** np.arange(POLY + 1)[None, :]
    T = np.arange(HALF, dtype=np.float64)[:, None] ** np.arange(POLY + 1)[None, :]
    E = T @ np.linalg.pinv(V)  # [HALF, W]
    return c, E


def _build_weights():
    """W matrices [128, 128] (stationary lhsT: W[q, p] multiplies input q for
    output p).  Returns (W_first, W_mid, W_last) float32."""
    c64, E64 = _savgol_matrices()
    c = c64.astype(np.float32)
    E = E64.astype(np.float32)

    w_mid = np.zeros((P, P), np.float32)
    for p in range(STRIDE):  # output p -> y[base + 5 + p] = sum_k c_k x[base+p+k]
        for k in range(WINDOW):
            w_mid[p + k, p] = c[k]

    w_first = np.zeros((P, P), np.float32)
    for p in range(HALF):  # y[p] = sum_w E[p, w] x[w]
        for w in range(WINDOW):
            w_first[w, p] = E[p, w]
    for p in range(HALF, EDGE_W):  # y[p] = sum_k c_k x[p - 5 + k]
        for k in range(WINDOW):
            w_first[p - HALF + k, p] = c[k]

    w_last = np.zeros((P, P), np.float32)
    for p in range(STRIDE):  # y[L-123+p] = sum_k c_k x[base + p + k]
        for k in range(WINDOW):
            w_last[p + k, p] = c[k]
    for h in range(HALF):  # y[L-5+h] = sum_w E[4-h, w] x[L-1-w] = x[base+127-w]
        for w in range(WINDOW):
            w_last[127 - w, STRIDE + h] = E[HALF - 1 - h, w]

    return w_first, w_mid, w_last


def _windows():
    """List of (in_off, out_off, out_width, wtype) per window; wtype in
    {'first','mid','last'}."""
    wins = [(0, 0, EDGE_W, "first")]
    j = 1
    while True:
        in_off = STRIDE * j
        out_off = in_off + HALF
        if out_off + STRIDE >= L - HALF:
            break
        wins.append((in_off, out_off, STRIDE, "mid"))
        j += 1
    wins.append((L - P, L - EDGE_W, EDGE_W, "last"))
    return wins


def reference_rows(x):
    """Numpy reference for [rows, L] (mirrors the jax reference)."""
    c64, E64 = _savgol_matrices()
    c = c64.astype(np.float32)
    E = E64.astype(np.float32)
    R, Lx = x.shape
    out = np.empty_like(x)
    # interior via correlation
    from numpy.lib.stride_tricks import sliding_window_view

    sw = sliding_window_view(x, WINDOW, axis=1)  # [R, L-10, 11]
    out[:, HALF : Lx - HALF] = np.einsum("rlk,k->rl", sw, c, optimize=True).astype(
        np.float32
    )
    out[:, :HALF] = x[:, :WINDOW] @ E.T
    out[:, Lx - HALF :] = (x[:, ::-1][:, :WINDOW] @ E.T)[:, ::-1]
    return out


def simulate_host(x):
    """Pure-numpy simulation of the windowed scheme, to validate W matrices."""
    w_first, w_mid, w_last = _build_weights()
    wmap = {"first": w_first, "mid": w_mid, "last": w_last}
    R, Lx = x.shape
    out = np.zeros_like(x)
    for in_off, out_off, width, t in _windows():
        xw = x[:, in_off : in_off + P]  # [R, 128]
        yw = xw @ wmap[t]  # [R, 128]
        out[:, out_off : out_off + width] = yw[:, :width]
    return out


# ---------------------------------------------------------------------------
# Bass kernel
# ---------------------------------------------------------------------------

IO_BATCH = 32  # windows per DMA in/out batch
CONV_GROUP = 4  # windows per conv matmul (moving dim 4*128=512)

_NC_CACHE = None
TRACE = False  # set by test harness to capture an NTFF profile
LAST_RESULTS = None  # BassKernelResults of the most recent kernel() call


XT_COPY_ENGINE = "vector"  # engine for PSUM->SBUF copy of transposed windows
Y_COPY_ENGINE = "scalar"  # engine for PSUM->SBUF copy of conv outputs


def _build_nc(
    reps: int = 1,
    n_batches: int | None = None,
    hw_loop: bool = False,
    ablate: frozenset = frozenset(),
):
    import concourse.tile as tile
    from concourse import bacc, mybir
    from concourse.masks import make_identity

    nc = bacc.Bacc(
        "TRN2",
        target_bir_lowering=False,
        debug=False,
        enable_asserts=False,
        num_devices=N_CORES,
    )
    bf16 = mybir.dt.bfloat16
    f32 = mybir.dt.float32
    x = nc.dram_tensor("x", [P, L], f32, kind="ExternalInput").ap()
    wf = nc.dram_tensor("w_first", [P, P], bf16, kind="ExternalInput").ap()
    wm = nc.dram_tensor("w_mid", [P, P], bf16, kind="ExternalInput").ap()
    wl = nc.dram_tensor("w_last", [P, P], bf16, kind="ExternalInput").ap()
    y = nc.dram_tensor("y", [P, L], f32, kind="ExternalOutput").ap()

    wins = _windows()
    batches = [wins[i : i + IO_BATCH] for i in range(0, len(wins), IO_BATCH)]
    if n_batches is not None:
        batches = batches[:n_batches]

    with tile.TileContext(nc) as tc:
        with ExitStack() as ctx:
            consts = ctx.enter_context(tc.tile_pool(name="consts", bufs=1))
            in_pool = ctx.enter_context(tc.tile_pool(name="inp", bufs=3))
            out_pool = ctx.enter_context(tc.tile_pool(name="outp", bufs=3))
            xt_pool = ctx.enter_context(tc.tile_pool(name="xt", bufs=4))
            ps_t = ctx.enter_context(tc.tile_pool(name="ps_t", bufs=4, space="PSUM"))
            ps_c = ctx.enter_context(tc.tile_pool(name="ps_c", bufs=4, space="PSUM"))

            ident = consts.tile([P, P], f32, tag="ident")
            make_identity(nc, ident[:])
            wt = {}
            for name, ap in (("first", wf), ("mid", wm), ("last", wl)):
                t = consts.tile([P, P], bf16, tag=f"w_{name}")
                nc.sync.dma_start(t[:], ap)
                wt[name] = t

            def copy(engine, dst, src):
                if engine == "scalar":
                    nc.scalar.copy(dst, src)
                else:
                    nc.vector.tensor_copy(dst, src)

            def body():
              for batch in batches:
                in_base = batch[0][0]
                in_span = batch[-1][0] + P - in_base
                out_base = batch[0][1]
                out_span = batch[-1][1] + batch[-1][2] - out_base

                xin = in_pool.tile([P, IO_BATCH * STRIDE + 256], f32, tag="xin")
                nc.sync.dma_start(
                    xin[:, :in_span], x[:, in_base : in_base + in_span]
                )
                stag = out_pool.tile([P, IO_BATCH * STRIDE + 256], f32, tag="stag")

                # group windows by consecutive same-type runs of <= CONV_GROUP
                groups = []
                cur = []
                for w in batch:
                    if cur and (w[3] != cur[0][3] or len(cur) == CONV_GROUP):
                        groups.append(cur)
                        cur = []
                    cur.append(w)
                groups.append(cur)

                for grp in groups:
                    g = len(grp)
                    wtype = grp[0][3]
                    # 1) PE transpose each window into one PSUM bank (fp32,
                    #    exact)
                    pt = ps_t.tile([P, 512], f32, tag="pt")
                    for s, (in_off, _, _, _) in enumerate(grp):
                        o = in_off - in_base
                        nc.tensor.transpose(
                            pt[:, s * P : (s + 1) * P],
                            xin[:, o : o + P],
                            ident[:],
                        )
                    # 2) copy PSUM -> SBUF, converting to bf16 (rel tol is
                    #    2e-2; bf16 conv gives ~4e-3)
                    xt = xt_pool.tile([P, 512], bf16, tag="xt")
                    copy(XT_COPY_ENGINE, xt[:, : g * P], pt[:, : g * P])
                    # 3) conv matmul per window in bf16 (1 cycle/row on PE vs
                    #    4 for fp32), stationary = transposed window, moving =
                    #    W -> output lands in NATURAL layout, packed
                    #    contiguously in the PSUM bank
                    width = grp[0][2]
                    pc = ps_c.tile([P, 512], f32, tag="pc")
                    for s in range(g):
                        nc.tensor.matmul(
                            pc[:, s * width : (s + 1) * width],
                            xt[:, s * P : (s + 1) * P],
                            wt[wtype][:, :width],
                            start=True,
                            stop=True,
                        )
                    # 4) single contiguous copy into the output staging buffer
                    soff = grp[0][1] - out_base
                    copy(Y_COPY_ENGINE, stag[:, soff : soff + g * width], pc[:, : g * width])

                nc.sync.dma_start(
                    y[:, out_base : out_base + out_span], stag[:, :out_span]
                )

            if hw_loop:
                with tc.For_i(0, reps):
                    body()
            else:
                for _rep in range(reps):
                    body()

    nc.compile()
    return nc


def _get_nc():
    global _NC_CACHE
    if _NC_CACHE is None:
        _NC_CACHE = _build_nc()
    return _NC_CACHE


def _make_in_maps(x: np.ndarray) -> list[dict]:
    import ml_dtypes

    w_first, w_mid, w_last = _build_weights()
    w_first = w_first.astype(ml_dtypes.bfloat16)
    w_mid = w_mid.astype(ml_dtypes.bfloat16)
    w_last = w_last.astype(ml_dtypes.bfloat16)
    xr = np.ascontiguousarray(x.reshape(ROWS_TOTAL, L))
    return [
        {
            "x": xr[i * ROWS_PER_CORE : (i + 1) * ROWS_PER_CORE],
            "w_first": w_first,
            "w_mid": w_mid,
            "w_last": w_last,
        }
        for i in range(N_CORES)
    ]


def kernel(x: np.ndarray) -> np.ndarray:
    from concourse.bass_utils import run_bass_kernel_spmd

    assert x.shape == (64, 16, L) and x.dtype == np.float32
    nc = _get_nc()
    in_maps = _make_in_maps(x)
    res = run_bass_kernel_spmd(
        nc, in_maps, core_ids=list(range(N_CORES)), trace=TRACE
    )
    globals()["LAST_RESULTS"] = res
    out = np.concatenate([r["y"] for r in res.results], axis=0)
    return out.reshape(64, 16, L)


if __name__ == "__main__":
    # host-side validation of the window scheme
    rng = np.random.default_rng(0)
    xt = rng.standard_normal((4, L)).astype(np.float32)
    ref = reference_rows(xt)
    sim = simulate_host(xt)
    err = np.abs(sim - ref).max()
    rel = err / np.abs(ref).max()
    print(f"host sim vs ref: max abs {err:.3e}  rel {rel:.3e}")
    print("n windows:", len(_windows()))

